# revision 1
# baseline (speedup 1.0000x reference)
"""Trainium2 Bass kernel for nn_LocalAttnDecoderRNN (1-token local-attn LSTM decoder).

Strategy (8 NeuronCores, SPMD):
  - Host extracts the single embedding row (emb[token]) and pre-packs every
    weight matrix into PE-friendly lhsT layouts (contraction dim on partitions).
  - attn_W / encoder_outputs sharded over source positions (512 per core),
    comb_W / LSTM sharded over the hidden dim (128 per core),
    out_W sharded over vocab (16000 rows per core).
  - Cross-core reductions via on-chip collectives: AllReduce for the attention
    center scalar / attn_applied vector / softmax sum-exp, AllGather for the
    comb output x and the residual output vector.
  - The dominant cost is streaming the out_W shard (65.5 MB/core) from HBM;
    everything else overlaps with that stream.
"""

import sys

sys.path.insert(0, "/opt/trn_rl_repo")

import numpy as np

import concourse.bass as bass  # noqa: F401  (bass types used indirectly)
import concourse.mybir as mybir
import concourse.tile as tile
from concourse import bacc, bass_utils
from concourse.bass_interp import get_hw_module

NCORES = 8
H = 1024
L = 4096
V = 128000
LSH = L // NCORES         # 512 positions per core
VSH = V // NCORES         # 16000 vocab rows per core
VB = VSH // 128           # 125 vocab blocks per core
WB = 5                    # vocab blocks per wo DMA tile
WT = VB // WB             # 25 wo DMA tiles
WIDTH = 5.0

F32 = mybir.dt.float32
AF = mybir.ActivationFunctionType
ALU = mybir.AluOpType

TRACE = False             # set by test harness to collect a HW profile
_BUILT = None             # cached compiled Bass module
LAST_RESULTS = None       # BassKernelResults of the most recent run


def _build():
    nc = bacc.Bacc("TRN2", target_bir_lowering=False, debug=False, num_devices=NCORES)

    def inp(name, shape):
        return nc.dram_tensor(name, shape, F32, kind="ExternalInput")

    x2h_d = inp("x2h", [128, 16])
    embT_d = inp("embT", [128, 8])
    hT_d = inp("hT", [128, 8])
    c0_d = inp("c0T", [128, 1])
    alb_d = inp("alb", [128, 1])
    aw_d = inp("attn_wt", [128, 16 * LSH])
    abt_d = inp("attn_bt", [128, 4])
    alw_d = inp("alwt", [128, 4])
    pos_d = inp("post", [128, 4])
    npos_d = inp("negpost", [128, 4])
    enc_d = inp("enc_t", [128, 4 * H])
    cw_d = inp("comb_wt", [128, 16 * 128])
    cb_d = inp("comb_bt", [128, 1])
    wih_d = inp("wih_t", [128, 32 * 128])
    whh_d = inp("whh_t", [128, 32 * 128])
    lb_d = inp("lstm_bt", [128, 4])
    wo_d = inp("wo_t", [128, VSH * 8])    # [kp, vb*1024 + kc*128 + j]
    ob_d = inp("out_bt", [128, VB])
    eye_d = inp("eye", [128, 128])
    ones_d = inp("ones", [128, 128])

    logp_d = nc.dram_tensor("logp_out", [128, VB], F32, kind="ExternalOutput")
    h_d = nc.dram_tensor("h_out", [128, 1], F32, kind="ExternalOutput")
    c_d = nc.dram_tensor("c_out", [128, 1], F32, kind="ExternalOutput")

    RG = [list(range(NCORES))]

    with tile.TileContext(nc) as tc:
        with (
            tc.tile_pool(name="const", bufs=1) as cp,
            tc.tile_pool(name="work", bufs=2) as wk,
            tc.tile_pool(name="wo", bufs=4) as wop,
            tc.tile_pool(name="psmm", bufs=4, space="PSUM") as pmm,
            tc.tile_pool(name="pst", bufs=2, space="PSUM") as pst,
            tc.tile_pool(name="dram", bufs=1, space="DRAM") as dp,
        ):
            def load(d, shape, tag):
                t = cp.tile(shape, F32, tag=tag)
                nc.sync.dma_start(t[:], d[:])
                return t

            # ---- small inputs first (they gate the early critical path) ----
            x2h_s = load(x2h_d, [128, 16], "x2h")
            embT_s = load(embT_d, [128, 8], "embT")
            hT_s = load(hT_d, [128, 8], "hT")
            c0_s = load(c0_d, [128, 1], "c0T")
            alb_s = load(alb_d, [128, 1], "alb")
            abt_s = load(abt_d, [128, 4], "abt")
            alw_s = load(alw_d, [128, 4], "alw")
            pos_s = load(pos_d, [128, 4], "pos")
            npos_s = load(npos_d, [128, 4], "npos")
            cb_s = load(cb_d, [128, 1], "cbt")
            lb_s = load(lb_d, [128, 4], "lbt")
            ob_s = load(ob_d, [128, VB], "obt")
            eye_s = load(eye_d, [128, 128], "eye")
            ones_s = load(ones_d, [128, 128], "ones")
            # ---- weights, in order of use ----
            aw_s = load(aw_d, [128, 16 * LSH], "aw")
            enc_s = load(enc_d, [128, 4 * H], "enc")
            cw_s = load(cw_d, [128, 16 * 128], "cw")
            wih_s = load(wih_d, [128, 32 * 128], "wih")
            whh_s = load(whh_d, [128, 32 * 128], "whh")

            logits_s = cp.tile([128, VB], F32, tag="logits")

            # =========== A: attention logits slice + center partial ===========
            # aw[l] = x2h @ attn_W[l,:]  for this core's 512 positions,
            # laid out [128, 4] partition-major (col m = positions m*128..).
            awt_s = cp.tile([128, 4], F32, tag="awt")
            for m in range(4):
                ps = pmm.tile([128, 1], F32, tag="pl")
                for kc in range(16):
                    nc.tensor.matmul(
                        ps[:],
                        aw_s[:, kc * LSH + m * 128: kc * LSH + (m + 1) * 128],
                        x2h_s[:, kc: kc + 1],
                        start=(kc == 0),
                        stop=(kc == 15),
                    )
                # add attn_b column while moving PSUM -> SBUF
                nc.vector.tensor_tensor(
                    awt_s[:, m: m + 1], ps[:], abt_s[:, m: m + 1], ALU.add
                )

            # center partial: sum_l aw[l] * attn_lin_W[l]  (this core's slice)
            mulc = wk.tile([128, 4], F32, tag="mulc")
            nc.vector.tensor_tensor(mulc[:], awt_s[:], alw_s[:], ALU.mult)
            redc = wk.tile([128, 1], F32, tag="redc")
            nc.vector.reduce_sum(redc[:], mulc[:], axis=mybir.AxisListType.X)
            psc = pmm.tile([128, 1], F32, tag="pl")
            nc.tensor.matmul(psc[:], ones_s[:], redc[:], start=True, stop=True)
            cpart = wk.tile([128, 1], F32, tag="cpart")
            nc.scalar.copy(cpart[:], psc[:])

            arc_i = dp.tile([128, 1], F32, tag="arc_i")
            arc_o = dp.tile([128, 1], F32, tag="arc_o")
            nc.sync.dma_start(arc_i[:], cpart[:])
            nc.gpsimd.collective_compute(
                "AllReduce", ALU.add, replica_groups=RG,
                ins=[arc_i[:].opt()], outs=[arc_o[:].opt()],
            )
            craw = wk.tile([128, 1], F32, tag="craw")
            nc.sync.dma_start(craw[:], arc_o[:])
            center = wk.tile([128, 1], F32, tag="center")
            nc.vector.tensor_tensor(center[:], craw[:], alb_s[:], ALU.add)

            # window: wv/1 = relu((p-c+W)/W) * relu((c-p+W)/W)
            s1b = wk.tile([128, 1], F32, tag="s1b")   # (W-c)/W = 1 - c/W
            nc.scalar.activation(s1b[:], center[:], AF.Identity, bias=1.0, scale=-1.0 / WIDTH)
            s2b = wk.tile([128, 1], F32, tag="s2b")   # (c+W)/W = 1 + c/W
            nc.scalar.activation(s2b[:], center[:], AF.Identity, bias=1.0, scale=1.0 / WIDTH)
            t1 = wk.tile([128, 4], F32, tag="t1")
            nc.scalar.activation(t1[:], pos_s[:], AF.Relu, bias=s1b[:], scale=1.0 / WIDTH)
            t2 = wk.tile([128, 4], F32, tag="t2")
            nc.scalar.activation(t2[:], npos_s[:], AF.Relu, bias=s2b[:], scale=1.0 / WIDTH)
            wv = wk.tile([128, 4], F32, tag="wv")
            nc.vector.tensor_tensor(wv[:], t1[:], t2[:], ALU.mult)
            la = wk.tile([128, 4], F32, tag="la")
            nc.vector.tensor_tensor(la[:], wv[:], awt_s[:], ALU.mult)

            # =========== B: attn_applied partial = la @ enc_slice ===========
            aab = wk.tile([128, 8], F32, tag="aab")
            for m in range(8):
                ps = pmm.tile([128, 1], F32, tag="pl")
                for kc in range(4):
                    nc.tensor.matmul(
                        ps[:],
                        enc_s[:, kc * H + m * 128: kc * H + (m + 1) * 128],
                        la[:, kc: kc + 1],
                        start=(kc == 0),
                        stop=(kc == 3),
                    )
                nc.scalar.copy(aab[:, m: m + 1], ps[:])

            ara_i = dp.tile([128, 8], F32, tag="ara_i")
            ara_o = dp.tile([128, 8], F32, tag="ara_o")
            nc.sync.dma_start(ara_i[:], aab[:])
            nc.gpsimd.collective_compute(
                "AllReduce", ALU.add, replica_groups=RG,
                ins=[ara_i[:].opt()], outs=[ara_o[:].opt()],
            )
            aaT = wk.tile([128, 8], F32, tag="aaT")
            nc.sync.dma_start(aaT[:], ara_o[:])

            # =========== C: comb + relu -> x slice; AllGather x ===========
            psx = pmm.tile([128, 1], F32, tag="pl")
            for kc in range(16):
                rhs = embT_s[:, kc: kc + 1] if kc < 8 else aaT[:, kc - 8: kc - 7]
                nc.tensor.matmul(
                    psx[:],
                    cw_s[:, kc * 128: (kc + 1) * 128],
                    rhs,
                    start=(kc == 0),
                    stop=(kc == 15),
                )
            xs = wk.tile([128, 1], F32, tag="xs")
            nc.scalar.activation(xs[:], psx[:], AF.Relu, bias=cb_s[:], scale=1.0)

            agx_i = dp.tile([128, 1], F32, tag="agx_i")
            agx_o = dp.tile([8, 128], F32, tag="agx_o")
            nc.sync.dma_start(agx_i[:], xs[:])
            nc.gpsimd.collective_compute(
                "AllGather", ALU.bypass, replica_groups=RG,
                ins=[agx_i[:].opt()], outs=[agx_o[:].opt()],
            )
            xg = cp.tile([128, 128], F32, tag="xg")
            nc.vector.memset(xg[:], 0.0)
            nc.sync.dma_start(xg[:8, :], agx_o[:])
            psxt = pst.tile([128, 128], F32, tag="pt")
            nc.tensor.transpose(psxt[:], xg[:], eye_s[:])
            xT = wk.tile([128, 8], F32, tag="xT")
            nc.scalar.copy(xT[:], psxt[:, :8])

            # =========== D: LSTM cell (hidden slice of 128) ===========
            gates = wk.tile([128, 4], F32, tag="gates")
            for g in range(4):
                ps = pmm.tile([128, 1], F32, tag="pl")
                for kc in range(8):
                    nc.tensor.matmul(
                        ps[:],
                        wih_s[:, (g * 8 + kc) * 128: (g * 8 + kc + 1) * 128],
                        xT[:, kc: kc + 1],
                        start=(kc == 0),
                        stop=False,
                    )
                for kc in range(8):
                    nc.tensor.matmul(
                        ps[:],
                        whh_s[:, (g * 8 + kc) * 128: (g * 8 + kc + 1) * 128],
                        hT_s[:, kc: kc + 1],
                        start=False,
                        stop=(kc == 7),
                    )
                nc.vector.tensor_tensor(
                    gates[:, g: g + 1], ps[:], lb_s[:, g: g + 1], ALU.add
                )

            sg01 = wk.tile([128, 2], F32, tag="sg01")
            nc.scalar.activation(sg01[:], gates[:, 0:2], AF.Sigmoid)
            tg = wk.tile([128, 1], F32, tag="tg")
            nc.scalar.activation(tg[:], gates[:, 2:3], AF.Tanh)
            so = wk.tile([128, 1], F32, tag="so")
            nc.scalar.activation(so[:], gates[:, 3:4], AF.Sigmoid)

            mf = wk.tile([128, 1], F32, tag="mf")
            nc.vector.tensor_tensor(mf[:], sg01[:, 1:2], c0_s[:], ALU.mult)
            mi = wk.tile([128, 1], F32, tag="mi")
            nc.vector.tensor_tensor(mi[:], sg01[:, 0:1], tg[:], ALU.mult)
            cn = wk.tile([128, 1], F32, tag="cn")
            nc.vector.tensor_tensor(cn[:], mf[:], mi[:], ALU.add)
            tcn = wk.tile([128, 1], F32, tag="tcn")
            nc.scalar.activation(tcn[:], cn[:], AF.Tanh)
            hn = wk.tile([128, 1], F32, tag="hn")
            nc.vector.tensor_tensor(hn[:], so[:], tcn[:], ALU.mult)
            osl = wk.tile([128, 1], F32, tag="osl")
            nc.vector.tensor_tensor(osl[:], hn[:], xs[:], ALU.add)

            nc.sync.dma_start(h_d[:], hn[:])
            nc.sync.dma_start(c_d[:], cn[:])

            ago_i = dp.tile([128, 1], F32, tag="ago_i")
            ago_o = dp.tile([8, 128], F32, tag="ago_o")
            nc.sync.dma_start(ago_i[:], osl[:])
            nc.gpsimd.collective_compute(
                "AllGather", ALU.bypass, replica_groups=RG,
                ins=[ago_i[:].opt()], outs=[ago_o[:].opt()],
            )
            og = cp.tile([128, 128], F32, tag="og")
            nc.vector.memset(og[:], 0.0)
            nc.sync.dma_start(og[:8, :], ago_o[:])
            psot = pst.tile([128, 128], F32, tag="pt")
            nc.tensor.transpose(psot[:], og[:], eye_s[:])
            oT = wk.tile([128, 8], F32, tag="oT")
            nc.scalar.copy(oT[:], psot[:, :8])

            # =========== E: the big out-projection stream ===========
            for t in range(WT):
                wt = wop.tile([128, WB * 8 * 128], F32, tag="wo")
                nc.sync.dma_start(
                    wt[:], wo_d[:, t * WB * 1024: (t + 1) * WB * 1024]
                )
                for vbl in range(WB):
                    vb = t * WB + vbl
                    ps = pmm.tile([128, 1], F32, tag="pl")
                    for kc in range(8):
                        nc.tensor.matmul(
                            ps[:],
                            wt[:, vbl * 1024 + kc * 128: vbl * 1024 + (kc + 1) * 128],
                            oT[:, kc: kc + 1],
                            start=(kc == 0),
                            stop=(kc == 7),
                        )
                    nc.scalar.copy(logits_s[:, vb: vb + 1], ps[:])

            # =========== F: log-softmax ===========
            nc.vector.tensor_tensor(logits_s[:], logits_s[:], ob_s[:], ALU.add)
            ex = cp.tile([128, VB], F32, tag="ex")
            nc.scalar.activation(ex[:], logits_s[:], AF.Exp)
            se = wk.tile([128, 1], F32, tag="se")
            nc.vector.reduce_sum(se[:], ex[:], axis=mybir.AxisListType.X)
            pss = pmm.tile([128, 1], F32, tag="pl")
            nc.tensor.matmul(pss[:], ones_s[:], se[:], start=True, stop=True)
            lsum = wk.tile([128, 1], F32, tag="lsum")
            nc.scalar.copy(lsum[:], pss[:])

            ars_i = dp.tile([128, 1], F32, tag="ars_i")
            ars_o = dp.tile([128, 1], F32, tag="ars_o")
            nc.sync.dma_start(ars_i[:], lsum[:])
            nc.gpsimd.collective_compute(
                "AllReduce", ALU.add, replica_groups=RG,
                ins=[ars_i[:].opt()], outs=[ars_o[:].opt()],
            )
            gs = wk.tile([128, 1], F32, tag="gs")
            nc.sync.dma_start(gs[:], ars_o[:])
            lz = wk.tile([128, 1], F32, tag="lz")
            nc.scalar.activation(lz[:], gs[:], AF.Ln)
            nc.vector.tensor_scalar(
                logits_s[:], logits_s[:], lz[:], None, ALU.subtract
            )
            nc.sync.dma_start(logp_d[:], logits_s[:])

    nc.compile()
    nc.m = get_hw_module(nc.m)
    return nc


def _prep_inputs(inputs):
    token = int(np.asarray(inputs["input_token"]).ravel()[0])
    emb = np.asarray(inputs["emb"], dtype=np.float32)
    h0 = np.asarray(inputs["h0"], dtype=np.float32).reshape(H)
    c0 = np.asarray(inputs["c0"], dtype=np.float32).reshape(H)
    enc = np.asarray(inputs["encoder_outputs"], dtype=np.float32)
    attn_W = np.asarray(inputs["attn_W"], dtype=np.float32)
    attn_b = np.asarray(inputs["attn_b"], dtype=np.float32)
    alw = np.asarray(inputs["attn_lin_W"], dtype=np.float32).reshape(L)
    albv = float(np.asarray(inputs["attn_lin_b"]).ravel()[0])
    comb_W = np.asarray(inputs["comb_W"], dtype=np.float32)
    comb_b = np.asarray(inputs["comb_b"], dtype=np.float32)
    W_ih = np.asarray(inputs["W_ih"], dtype=np.float32)
    W_hh = np.asarray(inputs["W_hh"], dtype=np.float32)
    lstm_b = (
        np.asarray(inputs["b_ih"], dtype=np.float32)
        + np.asarray(inputs["b_hh"], dtype=np.float32)
    )
    out_W = np.asarray(inputs["out_W"], dtype=np.float32)
    out_b = np.asarray(inputs["out_b"], dtype=np.float32)

    embr = emb[token]
    x2h = np.concatenate([embr, h0])
    common = {
        "x2h": np.ascontiguousarray(x2h.reshape(16, 128).T),
        "embT": np.ascontiguousarray(embr.reshape(8, 128).T),
        "hT": np.ascontiguousarray(h0.reshape(8, 128).T),
        "alb": np.full((128, 1), albv, dtype=np.float32),
        "eye": np.eye(128, dtype=np.float32),
        "ones": np.ones((128, 128), dtype=np.float32),
    }

    in_maps = []
    for cc in range(NCORES):
        sl = slice(cc * LSH, (cc + 1) * LSH)
        aw_sl = attn_W[sl]                                   # (512, 2048)
        awp = np.ascontiguousarray(
            aw_sl.reshape(LSH, 16, 128).transpose(2, 1, 0)
        ).reshape(128, 16 * LSH)
        enc_sl = enc[sl]                                     # (512, 1024)
        encp = np.ascontiguousarray(
            enc_sl.reshape(4, 128, H).transpose(1, 0, 2)
        ).reshape(128, 4 * H)
        comb_sl = comb_W[cc * 128: (cc + 1) * 128]           # (128, 2048)
        combp = np.ascontiguousarray(
            comb_sl.reshape(128, 16, 128).transpose(2, 1, 0)
        ).reshape(128, 16 * 128)

        def gate_pack(W):
            blocks = []
            for g in range(4):
                blk = W[g * H + cc * 128: g * H + (cc + 1) * 128]   # (128, 1024)
                blocks.append(
                    np.ascontiguousarray(
                        blk.reshape(128, 8, 128).transpose(2, 1, 0)
                    ).reshape(128, 8 * 128)
                )
            return np.ascontiguousarray(np.concatenate(blocks, axis=1))

        wihp = gate_pack(W_ih)
        whhp = gate_pack(W_hh)
        lbt = np.stack(
            [lstm_b[g * H + cc * 128: g * H + (cc + 1) * 128] for g in range(4)],
            axis=1,
        )

        Wo_sl = out_W[cc * VSH: (cc + 1) * VSH]              # (16000, 1024)
        wopk = np.ascontiguousarray(
            Wo_sl.reshape(VB, 128, 8, 128).transpose(3, 0, 2, 1)
        ).reshape(128, VSH * 8)

        pos = np.arange(cc * LSH, (cc + 1) * LSH, dtype=np.float32)
        m = {
            "c0T": np.ascontiguousarray(c0[cc * 128: (cc + 1) * 128].reshape(128, 1)),
            "attn_wt": awp,
            "attn_bt": np.ascontiguousarray(attn_b[sl].reshape(4, 128).T),
            "alwt": np.ascontiguousarray(alw[sl].reshape(4, 128).T),
            "post": np.ascontiguousarray(pos.reshape(4, 128).T),
            "negpost": np.ascontiguousarray((-pos).reshape(4, 128).T),
            "enc_t": encp,
            "comb_wt": combp,
            "comb_bt": np.ascontiguousarray(
                comb_b[cc * 128: (cc + 1) * 128].reshape(128, 1)
            ),
            "wih_t": wihp,
            "whh_t": whhp,
            "lstm_bt": np.ascontiguousarray(lbt),
            "wo_t": wopk,
            "out_bt": np.ascontiguousarray(
                out_b[cc * VSH: (cc + 1) * VSH].reshape(VB, 128).T
            ),
        }
        m.update(common)
        in_maps.append(m)
    return in_maps


def kernel(**inputs):
    global _BUILT, LAST_RESULTS
    if _BUILT is None:
        _BUILT = _build()
    nc = _BUILT
    in_maps = _prep_inputs(inputs)
    res = bass_utils.run_bass_kernel_spmd(
        nc, in_maps, core_ids=list(range(NCORES)), trace=TRACE
    )
    LAST_RESULTS = res
    out = res.results

    logp = np.concatenate(
        [out[c]["logp_out"].T.reshape(-1) for c in range(NCORES)]
    ).reshape(1, V)
    h_new = np.concatenate(
        [out[c]["h_out"].reshape(-1) for c in range(NCORES)]
    ).reshape(1, 1, H)
    c_new = np.concatenate(
        [out[c]["c_out"].reshape(-1) for c in range(NCORES)]
    ).reshape(1, 1, H)
    return (logp, h_new, c_new)
